# revision 5
# baseline (speedup 1.0000x reference)
"""Trainium2 Bass kernel for nn_CacheEvictionTransformer.

Sharding: data-parallel over batch. 16 items -> 8 NeuronCores x 2 items.
No collectives. Residual stream h lives in DRAM feature-major
[4 feature blocks][128][4224 tokens] (f32). All matmuls run feature-major:
LayerNorm stats and softmax sums are ones-matmul column reductions, so no
transposes are needed anywhere except the one-time embedding load.

Precision: f32r (FP22, single PE pass) for LN / projections / FFN-in /
out-proj; bf16 for attention score+AV operands and the ff2 matmul.
PSUM always accumulates fp32.

Causality: stream-a scores are computed per (row-chunk, key-tile) with the
masked-out column range skipped entirely; the single diagonal 128x128 block
is masked multiplicatively with a precomputed triangle after exp.
Cache rows (attend-everything) are processed as the last row chunk.
"""
import sys
sys.path.insert(0, "/opt/trn_rl_repo")
import contextlib
import numpy as np
import concourse.bass as bass
from concourse import bacc
import concourse.mybir as mybir
import concourse.tile as tile
from concourse.bass_utils import run_bass_kernel_spmd
from concourse.masks import make_identity

P = 128
B, K, W = 16, 64, 2048
D, DH, DFF, NL, V = 512, 256, 2048, 4, 32001
LN_EPS = 1e-5
L = K + W            # 2112 tokens per item
ITEMS = 2            # items per core
TT = ITEMS * L       # 4224 tokens per core
FB = D // P          # 4 feature blocks
DHB = DH // P        # 2
NCORES = 8
SCALE = float(1.0 / np.sqrt(np.float32(DH)))

F32 = mybir.dt.float32
F32R = mybir.dt.float32r
BF16 = mybir.dt.bfloat16
I32 = mybir.dt.int32
AF = mybir.ActivationFunctionType
OP = mybir.AluOpType

# per-item chunks: (chunk_id, col0 within item, ncols). chunk -1 = cache cols.
ITEM_CHUNKS = [(-1, 0, 64)] + [(c, 64 + 512 * c, 512) for c in range(4)]
GLOB_CHUNKS = [(512 * g, 512) for g in range(8)] + [(4096, 128)]

_CACHE = {}


def build_nc(n_layers=NL, debug_tap=None):
    """debug_tap: None | 'emb' | ('layer', i) -> extra output dbg_h with h."""
    nc = bacc.Bacc("TRN2", target_bir_lowering=False)

    ids = nc.declare_dram_parameter("ids", [TT, 1], I32, isOutput=False)
    emb = nc.declare_dram_parameter("item_embed", [V, D], F32, isOutput=False)
    cpos = nc.declare_dram_parameter("cache_pos", [K, D], F32, isOutput=False)
    spos = nc.declare_dram_parameter("seq_pos", [W, D], F32, isOutput=False)
    seg = nc.declare_dram_parameter("seg", [2, D], F32, isOutput=False)
    tri = nc.declare_dram_parameter("tri", [P, P], F32, isOutput=False)
    wq_s = nc.declare_dram_parameter("wq_s", [NL, D, DH], F32, isOutput=False)
    wk_s = nc.declare_dram_parameter("wk_s", [NL, D, DH], F32, isOutput=False)
    wv_s = nc.declare_dram_parameter("wv_s", [NL, D, DH], F32, isOutput=False)
    wq_c = nc.declare_dram_parameter("wq_c", [NL, D, DH], F32, isOutput=False)
    wk_c = nc.declare_dram_parameter("wk_c", [NL, D, DH], F32, isOutput=False)
    wv_c = nc.declare_dram_parameter("wv_c", [NL, D, DH], F32, isOutput=False)
    w_out = nc.declare_dram_parameter("w_out", [NL, D, D], F32, isOutput=False)
    b_out = nc.declare_dram_parameter("b_out", [NL, D], F32, isOutput=False)
    ln1_g = nc.declare_dram_parameter("ln1_g", [NL, D], F32, isOutput=False)
    ln1_b = nc.declare_dram_parameter("ln1_b", [NL, D], F32, isOutput=False)
    ln2_g = nc.declare_dram_parameter("ln2_g", [NL, D], F32, isOutput=False)
    ln2_b = nc.declare_dram_parameter("ln2_b", [NL, D], F32, isOutput=False)
    w_ff1 = nc.declare_dram_parameter("w_ff1", [NL, D, DFF], F32, isOutput=False)
    b_ff1 = nc.declare_dram_parameter("b_ff1", [NL, DFF], F32, isOutput=False)
    w_ff2 = nc.declare_dram_parameter("w_ff2", [NL, DFF, D], F32, isOutput=False)
    b_ff2 = nc.declare_dram_parameter("b_ff2", [NL, D], F32, isOutput=False)
    fin_g = nc.declare_dram_parameter("fin_g", [D], F32, isOutput=False)
    fin_b = nc.declare_dram_parameter("fin_b", [D], F32, isOutput=False)
    w_ev = nc.declare_dram_parameter("w_ev", [D, 1], F32, isOutput=False)
    b_ev = nc.declare_dram_parameter("b_ev", [1], F32, isOutput=False)
    out = nc.declare_dram_parameter("out", [ITEMS, K], F32, isOutput=True)
    if debug_tap is not None:
        dbg_h = nc.declare_dram_parameter("dbg_h", [FB, P, TT], F32, isOutput=True)

    with tile.TileContext(nc) as tc, contextlib.ExitStack() as ctx:
        consts = ctx.enter_context(tc.tile_pool(name="consts", bufs=1))
        vecs = ctx.enter_context(tc.tile_pool(name="vecs", bufs=2))
        wqkv = ctx.enter_context(tc.tile_pool(name="wqkv", bufs=1))
        wstage = ctx.enter_context(tc.tile_pool(name="wstage", bufs=2))
        mega = ctx.enter_context(tc.tile_pool(name="mega", bufs=1))
        hio = ctx.enter_context(tc.tile_pool(name="hio", bufs=2))
        hnp = ctx.enter_context(tc.tile_pool(name="hn", bufs=2))
        small = ctx.enter_context(tc.tile_pool(name="small", bufs=1))
        att = ctx.enter_context(tc.tile_pool(name="att", bufs=2))
        dram = ctx.enter_context(tc.tile_pool(name="dram", bufs=1, space="DRAM"))
        psum = ctx.enter_context(tc.tile_pool(name="psum", bufs=1, space="PSUM"))
        psum2 = ctx.enter_context(tc.tile_pool(name="psum2", bufs=2, space="PSUM"))

        hdram = dram.tile([FB, P, TT], F32, tag="hbuf")

        # ---------------- constants ----------------
        ones_col_f = consts.tile([P, 1], F32)
        nc.vector.memset(ones_col_f[:], 1.0)
        ones_col = consts.tile([P, 1], F32R)
        nc.vector.tensor_copy(out=ones_col[:], in_=ones_col_f[:])
        ones_col_b = consts.tile([P, 1], BF16)
        nc.vector.tensor_copy(out=ones_col_b[:], in_=ones_col_f[:])
        ones_row_f = consts.tile([1, P], F32)
        nc.vector.memset(ones_row_f[:], 1.0)
        ones_row = consts.tile([1, P], F32R)
        nc.vector.tensor_copy(out=ones_row[:], in_=ones_row_f[:])
        eps_t = consts.tile([1, 1], F32)
        nc.vector.memset(eps_t[:], LN_EPS)
        ident = consts.tile([P, P], F32)
        make_identity(nc, ident[:])
        tri_f = consts.tile([P, P], F32)
        nc.sync.dma_start(out=tri_f[:], in_=tri[:])
        tri_b = consts.tile([P, P], BF16)
        nc.vector.tensor_copy(out=tri_b[:], in_=tri_f[:])
        seg_sb = consts.tile([P, 2, FB], F32)
        nc.sync.dma_start(out=seg_sb[:], in_=seg.rearrange("s (fb p) -> p s fb", p=P))
        bev_t = consts.tile([1, 1], F32)
        nc.sync.dma_start(out=bev_t[:], in_=b_ev[:, None])

        # ---------------- helpers ----------------
        def load_h_chunk(gcol0, n):
            hf = hio.tile([P, FB, n], F32, tag="hf")
            for fb in range(FB):
                nc.sync.dma_start(out=hf[:, fb, :], in_=hdram[fb, :, gcol0:gcol0 + n])
            return hf

        def emit_ln(hf, n, g_vec, b_vec):
            """LN over feature dim for one chunk; returns hn [P, FB, n] f32r."""
            mu_ps = psum.tile([1, n], F32, tag="stat1")
            msq_ps = psum.tile([1, n], F32, tag="stat2")
            for fb in range(FB):
                hrf = small.tile([P, n], F32R, tag="hrf")
                nc.scalar.activation(out=hrf[:], in_=hf[:, fb, :], func=AF.Copy)
                nc.tensor.matmul(out=mu_ps[:], lhsT=ones_col[:], rhs=hrf[:],
                                 start=(fb == 0), stop=(fb == FB - 1))
                hsq = small.tile([P, n], F32R, tag="hsq")
                nc.scalar.activation(out=hsq[:], in_=hrf[:], func=AF.Square)
                nc.tensor.matmul(out=msq_ps[:], lhsT=ones_col[:], rhs=hsq[:],
                                 start=(fb == 0), stop=(fb == FB - 1))
            mu = small.tile([1, n], F32R, tag="mu")
            msq = small.tile([1, n], F32, tag="msq")
            nc.scalar.activation(out=mu[:], in_=mu_ps[:], func=AF.Copy, scale=1.0 / D)
            nc.scalar.activation(out=msq[:], in_=msq_ps[:], func=AF.Copy, scale=1.0 / D)
            var = small.tile([1, n], F32, tag="var")
            nc.vector.tensor_tensor(out=var[:], in0=mu[:], in1=mu[:], op=OP.mult)
            nc.vector.tensor_tensor(out=var[:], in0=msq[:], in1=var[:], op=OP.subtract)
            rstd = small.tile([1, n], F32R, tag="rstd")
            nc.scalar.activation(out=rstd[:], in_=var[:], func=AF.Sqrt, bias=eps_t[:])
            with nc.allow_low_precision(reason="rstd f32r feeds broadcast matmul"):
                nc.vector.reciprocal(out=rstd[:], in_=rstd[:])
            bc_mu = psum.tile([P, n], F32, tag="bcA")
            bc_r = psum.tile([P, n], F32, tag="bcB")
            nc.tensor.matmul(out=bc_mu[:], lhsT=ones_row[:], rhs=mu[:], start=True, stop=True)
            nc.tensor.matmul(out=bc_r[:], lhsT=ones_row[:], rhs=rstd[:], start=True, stop=True)
            hn = hnp.tile([P, FB, n], F32R, tag="hn")
            for fb in range(FB):
                tmp = small.tile([P, n], F32, tag="lntmp")
                nc.vector.tensor_tensor(out=tmp[:], in0=hf[:, fb, :], in1=bc_mu[:], op=OP.subtract)
                nc.vector.tensor_tensor(out=tmp[:], in0=tmp[:], in1=bc_r[:], op=OP.mult)
                nc.vector.tensor_scalar(out=hn[:, fb, :], in0=tmp[:],
                                        scalar1=g_vec[:, fb:fb + 1], scalar2=b_vec[:, fb:fb + 1],
                                        op0=OP.mult, op1=OP.add)
            return hn

        def writeback_fb(fb, gcol0, n, d_ps, bias_vec):
            """h[fb, :, cols] += d_ps + bias."""
            hres = hio.tile([P, n], F32, tag="hres")
            nc.sync.dma_start(out=hres[:], in_=hdram[fb, :, gcol0:gcol0 + n])
            hnew = hio.tile([P, n], F32, tag="hnew")
            nc.vector.tensor_scalar_add(out=hnew[:], in0=d_ps[:],
                                        scalar1=bias_vec[:, fb:fb + 1])
            nc.vector.tensor_tensor(out=hnew[:], in0=hnew[:], in1=hres[:], op=OP.add)
            nc.sync.dma_start(out=hdram[fb, :, gcol0:gcol0 + n], in_=hnew[:])

        def load_weight(dram_ap, kdim, mdim, tag, pool, dt=F32R):
            """DMA (kdim, mdim) weight -> [P, kdim//P, mdim] rounded tile."""
            ko = kdim // P
            wr = pool.tile([P, ko, mdim], dt, tag=tag)
            src = dram_ap.rearrange("(ko p) m -> p ko m", p=P)
            mstep = max(1, (2048 // max(ko, 1)))  # stage pieces <= [P, ko, mstep] (8KB)
            m0 = 0
            while m0 < mdim:
                m1 = min(m0 + mstep, mdim)
                wf = wstage.tile([P, ko, m1 - m0], F32, tag="stage")
                nc.sync.dma_start(out=wf[:], in_=src[:, :, m0:m1])
                if dt == F32R:
                    nc.gpsimd.tensor_copy(out=wr[:, :, m0:m1], in_=wf[:])
                else:
                    nc.vector.tensor_copy(out=wr[:, :, m0:m1], in_=wf[:])
                m0 = m1
            return wr

        def load_vec(dram_ap, nblk, tag):
            v = vecs.tile([P, nblk], F32, tag=tag)
            nc.sync.dma_start(out=v[:], in_=dram_ap.rearrange("(nb p) -> p nb", p=P))
            return v

        # ---------------- embedding ----------------
        for tt in range(TT // P):
            col0 = tt * P
            idx_sb = att.tile([P, 1], I32, tag="idx")
            nc.sync.dma_start(out=idx_sb[:], in_=ids[col0:col0 + P, :])
            g = att.tile([P, D], F32, tag="qa")
            nc.gpsimd.indirect_dma_start(
                out=g[:], out_offset=None, in_=emb[:],
                in_offset=bass.IndirectOffsetOnAxis(ap=idx_sb[:, :1], axis=0))
            ptile = att.tile([P, D], F32, tag="qb")
            r = 0
            while r < P:
                tok = col0 + r
                it, off = divmod(tok, L)
                if off < K:
                    ln = min(K - off, P - r)
                    nc.sync.dma_start(out=ptile[r:r + ln, :], in_=cpos[off:off + ln, :])
                else:
                    j = off - K
                    ln = min(W - j, P - r)
                    nc.sync.dma_start(out=ptile[r:r + ln, :], in_=spos[j:j + ln, :])
                r += ln
            nc.vector.tensor_tensor(out=g[:], in0=g[:], in1=ptile[:], op=OP.add)
            for fb in range(FB):
                tp = psum2.tile([P, P], F32, tag="mm")
                nc.tensor.transpose(out=tp[:], in_=g[:, fb * P:(fb + 1) * P], identity=ident[:])
                hout = att.tile([P, P], F32, tag="et")
                r = 0
                while r < P:
                    tok = col0 + r
                    it, off = divmod(tok, L)
                    s = 0 if off < K else 1
                    ln = (min(K - off, P - r)) if off < K else (min(L - off, P - r))
                    nc.vector.tensor_scalar_add(out=hout[:, r:r + ln], in0=tp[:, r:r + ln],
                                                scalar1=seg_sb[:, s, fb:fb + 1])
                    r += ln
                nc.sync.dma_start(out=hdram[fb, :, col0:col0 + P], in_=hout[:])

        if debug_tap == "emb":
            for fb in range(FB):
                nc.sync.dma_start(out=dbg_h[fb, :, :], in_=hdram[fb, :, :])

        # ---------------- transformer layers ----------------
        for l in range(n_layers):
            wq_s_r = load_weight(wq_s[l], D, DH, "wq_s", wqkv)
            wk_s_r = load_weight(wk_s[l], D, DH, "wk_s", wqkv)
            wv_s_r = load_weight(wv_s[l], D, DH, "wv_s", wqkv)
            wq_c_r = load_weight(wq_c[l], D, DH, "wq_c", wqkv)
            wk_c_r = load_weight(wk_c[l], D, DH, "wk_c", wqkv)
            wv_c_r = load_weight(wv_c[l], D, DH, "wv_c", wqkv)
            wo_r = load_weight(w_out[l], D, D, "wout", wqkv)
            g1 = load_vec(ln1_g[l], FB, "ln1g")
            b1 = load_vec(ln1_b[l], FB, "ln1b")
            bo = load_vec(b_out[l], FB, "bout")

            for item in range(ITEMS):
                base = item * L
                kaT = mega.tile([P, DHB, W], BF16, tag="mA")
                va = mega.tile([P, W // P, DH], BF16, tag="mB")
                kbT = att.tile([P, DHB, K], BF16, tag="kbT")
                vb = att.tile([P, DH], BF16, tag="vb")
                qc_a = att.tile([P, DHB, K], BF16, tag="qc_a")
                qc_b = att.tile([P, DHB, K], BF16, tag="qc_b")

                def project(hn, n, w_r, mdim, slice_fn):
                    for mo in range(mdim // P):
                        pj = psum2.tile([P, n], F32, tag="mm")
                        for ko in range(FB):
                            nc.tensor.matmul(
                                out=pj[:], lhsT=w_r[:, ko, mo * P:(mo + 1) * P],
                                rhs=hn[:, ko, :], start=(ko == 0), stop=(ko == FB - 1))
                        nc.vector.tensor_copy(out=slice_fn(mo), in_=pj[:])

                def attn_chunk(c, col0, n, hn_or_none):
                    if c == -1:
                        qa, qb = qc_a, qc_b
                    else:
                        qa = att.tile([P, DHB, n], BF16, tag="qa")
                        qb = att.tile([P, DHB, n], BF16, tag="qb")
                        project(hn_or_none, n, wq_s_r, DH, lambda mo: qa[:, mo, :])
                        project(hn_or_none, n, wq_c_r, DH, lambda mo: qb[:, mo, :])
                    n_kt = 16 if c == -1 else 4 * c + 4
                    oa0 = psum.tile([P, n], F32, tag="oa0")
                    oa1 = psum.tile([P, n], F32, tag="oa1")
                    suma = psum.tile([1, n], F32, tag="stat1")
                    for kt in range(n_kt):
                        lo = 0 if (c == -1 or kt < 4 * c) else 128 * (kt - 4 * c)
                        s_ps = psum2.tile([P, n - lo], F32, tag="mm")
                        for dhb in range(DHB):
                            nc.tensor.matmul(
                                out=s_ps[:], lhsT=kaT[:, dhb, kt * P:(kt + 1) * P],
                                rhs=qa[:, dhb, lo:n], start=(dhb == 0), stop=(dhb == DHB - 1))
                        e_t = att.tile([P, n], BF16, tag="et")
                        nc.scalar.activation(out=e_t[:, lo:n], in_=s_ps[:], func=AF.Exp,
                                             scale=SCALE)
                        if c != -1 and kt >= 4 * c:
                            nc.vector.tensor_tensor(out=e_t[:, lo:lo + P], in0=e_t[:, lo:lo + P],
                                                    in1=tri_b[:], op=OP.mult)
                        st, sp = kt == 0, kt == n_kt - 1
                        nc.tensor.matmul(out=suma[:, lo:n], lhsT=ones_col_b[:],
                                         rhs=e_t[:, lo:n], start=st, stop=sp)
                        nc.tensor.matmul(out=oa0[:, lo:n], lhsT=va[:, kt, 0:P],
                                         rhs=e_t[:, lo:n], start=st, stop=sp)
                        nc.tensor.matmul(out=oa1[:, lo:n], lhsT=va[:, kt, P:DH],
                                         rhs=e_t[:, lo:n], start=st, stop=sp)
                    # stream b (cache keys, full attention)
                    sb_ps = psum2.tile([K, n], F32, tag="mm")
                    for dhb in range(DHB):
                        nc.tensor.matmul(out=sb_ps[:], lhsT=kbT[:, dhb, :], rhs=qb[:, dhb, :],
                                         start=(dhb == 0), stop=(dhb == DHB - 1))
                    e_b = att.tile([K, n], BF16, tag="eb")
                    nc.scalar.activation(out=e_b[:], in_=sb_ps[:], func=AF.Exp, scale=SCALE)
                    sumb = psum.tile([1, n], F32, tag="stat2")
                    nc.tensor.matmul(out=sumb[:], lhsT=ones_col_b[:K, :], rhs=e_b[:],
                                     start=True, stop=True)
                    ob0 = psum2.tile([P, n], F32, tag="mm")
                    ob1 = psum2.tile([P, n], F32, tag="mm")
                    nc.tensor.matmul(out=ob0[:], lhsT=vb[:K, 0:P], rhs=e_b[:], start=True, stop=True)
                    nc.tensor.matmul(out=ob1[:], lhsT=vb[:K, P:DH], rhs=e_b[:], start=True, stop=True)
                    # normalize + concat
                    ra = small.tile([1, n], F32R, tag="ra")
                    rb = small.tile([1, n], F32R, tag="rb")
                    with nc.allow_low_precision(reason="softmax recip f32r feeds broadcast matmul"):
                        nc.vector.reciprocal(out=ra[:], in_=suma[:])
                        nc.vector.reciprocal(out=rb[:], in_=sumb[:])
                    bca_ps = psum.tile([P, n], F32, tag="bcA")
                    bcb_ps = psum.tile([P, n], F32, tag="bcB")
                    nc.tensor.matmul(out=bca_ps[:], lhsT=ones_row[:], rhs=ra[:], start=True, stop=True)
                    nc.tensor.matmul(out=bcb_ps[:], lhsT=ones_row[:], rhs=rb[:], start=True, stop=True)
                    bca = small.tile([P, n], F32, tag="bca")
                    bcb = small.tile([P, n], F32, tag="bcb")
                    nc.vector.tensor_copy(out=bca[:], in_=bca_ps[:])
                    nc.vector.tensor_copy(out=bcb[:], in_=bcb_ps[:])
                    ao = mega.tile([P, FB, n], F32R, tag="mI")
                    nc.vector.tensor_tensor(out=ao[:, 0, :], in0=oa0[:], in1=bca[:], op=OP.mult)
                    nc.vector.tensor_tensor(out=ao[:, 1, :], in0=oa1[:], in1=bca[:], op=OP.mult)
                    nc.vector.tensor_tensor(out=ao[:, 2, :], in0=ob0[:], in1=bcb[:], op=OP.mult)
                    nc.vector.tensor_tensor(out=ao[:, 3, :], in0=ob1[:], in1=bcb[:], op=OP.mult)
                    for fb in range(FB):
                        dp = psum2.tile([P, n], F32, tag="mm")
                        for ko in range(FB):
                            nc.tensor.matmul(out=dp[:], lhsT=wo_r[:, ko, fb * P:(fb + 1) * P],
                                             rhs=ao[:, ko, :], start=(ko == 0), stop=(ko == FB - 1))
                        writeback_fb(fb, base + col0, n, dp, bo)

                for (c, col0, n) in ITEM_CHUNKS:
                    hf = load_h_chunk(base + col0, n)
                    hn = emit_ln(hf, n, g1, b1)
                    if c == -1:
                        project(hn, n, wk_c_r, DH, lambda mo: kbT[:, mo, :])
                        project(hn, n, wq_s_r, DH, lambda mo: qc_a[:, mo, :])
                        project(hn, n, wq_c_r, DH, lambda mo: qc_b[:, mo, :])
                        vb_ps = psum2.tile([K, DH], F32, tag="mm")
                        for ko in range(FB):
                            nc.tensor.matmul(out=vb_ps[:], lhsT=hn[:, ko, :], rhs=wv_c_r[:, ko, :],
                                             start=(ko == 0), stop=(ko == FB - 1))
                        nc.vector.tensor_copy(out=vb[:K, :], in_=vb_ps[:])
                    else:
                        project(hn, n, wk_s_r, DH,
                                lambda mo: kaT[:, mo, 512 * c:512 * c + n])
                        for i in range(4):
                            kt = 4 * c + i
                            va_ps = psum2.tile([P, DH], F32, tag="mm")
                            for ko in range(FB):
                                nc.tensor.matmul(out=va_ps[:], lhsT=hn[:, ko, i * P:(i + 1) * P],
                                                 rhs=wv_s_r[:, ko, :], start=(ko == 0), stop=(ko == FB - 1))
                            nc.vector.tensor_copy(out=va[:, kt, :], in_=va_ps[:])
                        attn_chunk(c, col0, n, hn)
                attn_chunk(-1, 0, 64, None)

            # ---- FFN ----
            wf1h = []
            for h in range(2):
                w1h = load_weight(w_ff1[l][:, h * 1024:(h + 1) * 1024], D, 1024,
                                  ("mA", "mB")[h], mega)
                wf1h.append(w1h)
            wf2h = []
            for h in range(2):
                w2h = load_weight(w_ff2[l][h * 1024:(h + 1) * 1024, :], 1024, D,
                                  ("mC", "mD")[h], mega, dt=BF16)
                wf2h.append(w2h)
            g2 = load_vec(ln2_g[l], FB, "ln2g")
            b2 = load_vec(ln2_b[l], FB, "ln2b")
            bf1 = load_vec(b_ff1[l], DFF // P, "bff1")
            bf2 = load_vec(b_ff2[l], FB, "bff2")

            for (gc0, n) in GLOB_CHUNKS:
                hf = load_h_chunk(gc0, n)
                hn2 = emit_ln(hf, n, g2, b2)
                ffT = [mega.tile([P, 8, n], BF16, tag=t, name=f"ffT_{t}") for t in ("mI", "mF")]
                for h in range(2):
                    for mo in range(8):
                        fp = psum2.tile([P, n], F32, tag="mm")
                        for ko in range(FB):
                            nc.tensor.matmul(out=fp[:], lhsT=wf1h[h][:, ko, mo * P:(mo + 1) * P],
                                             rhs=hn2[:, ko, :], start=(ko == 0), stop=(ko == FB - 1))
                        nc.scalar.activation(out=ffT[h][:, mo, :], in_=fp[:], func=AF.Relu,
                                             bias=bf1[:, 8 * h + mo:8 * h + mo + 1])
                for fb in range(FB):
                    dp = psum2.tile([P, n], F32, tag="mm")
                    first = True
                    for h in range(2):
                        for ko in range(8):
                            nc.tensor.matmul(out=dp[:], lhsT=wf2h[h][:, ko, fb * P:(fb + 1) * P],
                                             rhs=ffT[h][:, ko, :], start=first,
                                             stop=(h == 1 and ko == 7))
                            first = False
                    writeback_fb(fb, gc0, n, dp, bf2)

            if debug_tap == ("layer", l):
                for fb in range(FB):
                    nc.sync.dma_start(out=dbg_h[fb, :, :], in_=hdram[fb, :, :])

        # ---------------- final LN + logits ----------------
        gF = load_vec(fin_g, FB, "fing")
        bF = load_vec(fin_b, FB, "finb")
        wev_r = load_weight(w_ev, D, 1, "wev", wqkv)
        for item in range(ITEMS):
            hf = load_h_chunk(item * L, K)
            hnF = emit_ln(hf, K, gF, bF)
            lg = psum2.tile([1, K], F32, tag="mm")
            for ko in range(FB):
                nc.tensor.matmul(out=lg[:], lhsT=wev_r[:, ko, :], rhs=hnF[:, ko, :],
                                 start=(ko == 0), stop=(ko == FB - 1))
            o_sb = small.tile([1, K], F32, tag="osb")
            nc.vector.tensor_scalar_add(out=o_sb[:], in0=lg[:], scalar1=bev_t[:])
            nc.sync.dma_start(out=out[item:item + 1, :], in_=o_sb[:])

    nc.finalize()
    return nc


def make_in_maps(inputs):
    f32 = lambda x: np.ascontiguousarray(np.asarray(x), dtype=np.float32)
    cache = np.asarray(inputs["cache"]).astype(np.int32)
    seq = np.asarray(inputs["seq"]).astype(np.int32)
    shared = {
        "item_embed": f32(inputs["item_embed"]),
        "cache_pos": f32(inputs["cache_pos_embed"]),
        "seq_pos": f32(inputs["seq_pos_embed"]),
        "seg": f32(inputs["segment_embed"]),
        "tri": np.triu(np.ones((P, P), np.float32)),
        "w_out": f32(inputs["w_out"]), "b_out": f32(inputs["b_out"]),
        "ln1_g": f32(inputs["ln1_g"]), "ln1_b": f32(inputs["ln1_b"]),
        "ln2_g": f32(inputs["ln2_g"]), "ln2_b": f32(inputs["ln2_b"]),
        "w_ff1": f32(inputs["w_ff1"]), "b_ff1": f32(inputs["b_ff1"]),
        "w_ff2": f32(inputs["w_ff2"]), "b_ff2": f32(inputs["b_ff2"]),
        "fin_g": f32(inputs["fin_g"]), "fin_b": f32(inputs["fin_b"]),
        "w_ev": f32(inputs["w_ev"]), "b_ev": f32(inputs["b_ev"]),
        "wq_s": f32(inputs["wq_s"]), "wk_s": f32(inputs["wk_s"]),
        "wv_s": f32(inputs["wv_s"]), "wq_c": f32(inputs["wq_c"]),
        "wk_c": f32(inputs["wk_c"]), "wv_c": f32(inputs["wv_c"]),
    }
    in_maps = []
    for core in range(NCORES):
        ids_l = []
        for item in range(ITEMS):
            b = core * ITEMS + item
            ids_l.append(np.concatenate([cache[b], seq[b]]))
        m = dict(shared)
        m["ids"] = np.concatenate(ids_l).reshape(TT, 1).astype(np.int32)
        in_maps.append(m)
    return in_maps


def kernel(**inputs) -> np.ndarray:
    if "nc" not in _CACHE:
        _CACHE["nc"] = build_nc()
    res = run_bass_kernel_spmd(_CACHE["nc"], make_in_maps(inputs), list(range(NCORES)))
    outs = [res.results[c]["out"] for c in range(NCORES)]
    return np.concatenate(outs, axis=0).astype(np.float32)


if __name__ == "__main__":
    import time
    t0 = time.time()
    build_nc()
    print(f"build+finalize: {time.time()-t0:.1f}s")


# revision 8
# speedup vs baseline: 240.0121x; 240.0121x over previous
"""Trainium2 Bass kernel for nn_CacheEvictionTransformer.

Sharding: data-parallel over batch. 16 items -> 8 NeuronCores x 2 items.
No collectives. Residual stream h lives in DRAM feature-major
[4 feature blocks][128][4224 tokens] (f32). All matmuls run feature-major:
LayerNorm stats and softmax sums are ones-matmul column reductions, so no
transposes are needed anywhere except the one-time embedding load.

Precision: f32r (FP22, single PE pass) for LN / projections / FFN-in /
out-proj; bf16 for attention score+AV operands and the ff2 matmul.
PSUM always accumulates fp32.

Causality: stream-a scores are computed per (row-chunk, key-tile) with the
masked-out column range skipped entirely; the single diagonal 128x128 block
is masked multiplicatively with a precomputed triangle after exp.
Cache rows (attend-everything) are processed as the last row chunk.
"""
import sys
sys.path.insert(0, "/opt/trn_rl_repo")
import contextlib
import numpy as np
import concourse.bass as bass
from concourse import bacc
import concourse.mybir as mybir
import concourse.tile as tile
from concourse.bass_utils import run_bass_kernel_spmd
from concourse.masks import make_identity

P = 128
B, K, W = 16, 64, 2048
D, DH, DFF, NL, V = 512, 256, 2048, 4, 32001
LN_EPS = 1e-5
L = K + W            # 2112 tokens per item
ITEMS = 2            # items per core
TT = ITEMS * L       # 4224 tokens per core
FB = D // P          # 4 feature blocks
DHB = DH // P        # 2
NCORES = 8
SCALE = float(1.0 / np.sqrt(np.float32(DH)))

F32 = mybir.dt.float32
F32R = mybir.dt.float32r
BF16 = mybir.dt.bfloat16
I32 = mybir.dt.int32
AF = mybir.ActivationFunctionType
OP = mybir.AluOpType

# per-item chunks: (chunk_id, col0 within item, ncols). chunk -1 = cache cols.
ITEM_CHUNKS = [(-1, 0, 64)] + [(c, 64 + 512 * c, 512) for c in range(4)]
GLOB_CHUNKS = [(512 * g, 512) for g in range(8)] + [(4096, 128)]

_CACHE = {}


def build_nc(n_layers=NL, debug_tap=None):
    """debug_tap: None | 'emb' | ('layer', i) -> extra output dbg_h with h."""
    nc = bacc.Bacc("TRN2", target_bir_lowering=False)

    ids = nc.declare_dram_parameter("ids", [TT, 1], I32, isOutput=False)
    emb = nc.declare_dram_parameter("item_embed", [V, D], F32, isOutput=False)
    cpos = nc.declare_dram_parameter("cache_pos", [K, D], F32, isOutput=False)
    spos = nc.declare_dram_parameter("seq_pos", [W, D], F32, isOutput=False)
    seg = nc.declare_dram_parameter("seg", [2, D], F32, isOutput=False)
    tri = nc.declare_dram_parameter("tri", [P, P], F32, isOutput=False)
    wq_s = nc.declare_dram_parameter("wq_s", [NL, D, DH], F32, isOutput=False)
    wk_s = nc.declare_dram_parameter("wk_s", [NL, D, DH], F32, isOutput=False)
    wv_s = nc.declare_dram_parameter("wv_s", [NL, D, DH], F32, isOutput=False)
    wq_c = nc.declare_dram_parameter("wq_c", [NL, D, DH], F32, isOutput=False)
    wk_c = nc.declare_dram_parameter("wk_c", [NL, D, DH], F32, isOutput=False)
    wv_c = nc.declare_dram_parameter("wv_c", [NL, D, DH], F32, isOutput=False)
    w_out = nc.declare_dram_parameter("w_out", [NL, D, D], F32, isOutput=False)
    b_out = nc.declare_dram_parameter("b_out", [NL, D], F32, isOutput=False)
    ln1_g = nc.declare_dram_parameter("ln1_g", [NL, D], F32, isOutput=False)
    ln1_b = nc.declare_dram_parameter("ln1_b", [NL, D], F32, isOutput=False)
    ln2_g = nc.declare_dram_parameter("ln2_g", [NL, D], F32, isOutput=False)
    ln2_b = nc.declare_dram_parameter("ln2_b", [NL, D], F32, isOutput=False)
    w_ff1 = nc.declare_dram_parameter("w_ff1", [NL, D, DFF], F32, isOutput=False)
    b_ff1 = nc.declare_dram_parameter("b_ff1", [NL, DFF], F32, isOutput=False)
    w_ff2 = nc.declare_dram_parameter("w_ff2", [NL, DFF, D], F32, isOutput=False)
    b_ff2 = nc.declare_dram_parameter("b_ff2", [NL, D], F32, isOutput=False)
    fin_g = nc.declare_dram_parameter("fin_g", [D], F32, isOutput=False)
    fin_b = nc.declare_dram_parameter("fin_b", [D], F32, isOutput=False)
    w_ev = nc.declare_dram_parameter("w_ev", [D, 1], F32, isOutput=False)
    b_ev = nc.declare_dram_parameter("b_ev", [1], F32, isOutput=False)
    out = nc.declare_dram_parameter("out", [ITEMS, K], F32, isOutput=True)
    if debug_tap is not None:
        dbg_h = nc.declare_dram_parameter("dbg_h", [FB, P, TT], F32, isOutput=True)

    with tile.TileContext(nc) as tc, contextlib.ExitStack() as ctx:
        consts = ctx.enter_context(tc.tile_pool(name="consts", bufs=1))
        vecs = ctx.enter_context(tc.tile_pool(name="vecs", bufs=2))
        wqkv = ctx.enter_context(tc.tile_pool(name="wqkv", bufs=1))
        wstage = ctx.enter_context(tc.tile_pool(name="wstage", bufs=1))
        mega = ctx.enter_context(tc.tile_pool(name="mega", bufs=1))
        hio = ctx.enter_context(tc.tile_pool(name="hio", bufs=2))
        hnp = ctx.enter_context(tc.tile_pool(name="hn", bufs=2))
        small = ctx.enter_context(tc.tile_pool(name="small", bufs=1))
        att = ctx.enter_context(tc.tile_pool(name="att", bufs=2))
        dram = ctx.enter_context(tc.tile_pool(name="dram", bufs=1, space="DRAM"))
        psum = ctx.enter_context(tc.tile_pool(name="psum", bufs=1, space="PSUM"))
        psum2 = ctx.enter_context(tc.tile_pool(name="psum2", bufs=2, space="PSUM"))

        hdram = dram.tile([FB, P, TT], F32, tag="hbuf")

        # ---------------- constants ----------------
        ones_col_f = consts.tile([P, 1], F32)
        nc.vector.memset(ones_col_f[:], 1.0)
        ones_col = consts.tile([P, 1], F32R)
        nc.vector.tensor_copy(out=ones_col[:], in_=ones_col_f[:])
        ones_col_b = consts.tile([P, 1], BF16)
        nc.vector.tensor_copy(out=ones_col_b[:], in_=ones_col_f[:])
        ones_row_f = consts.tile([1, P], F32)
        nc.vector.memset(ones_row_f[:], 1.0)
        ones_row = consts.tile([1, P], F32R)
        nc.vector.tensor_copy(out=ones_row[:], in_=ones_row_f[:])
        eps_t = consts.tile([1, 1], F32)
        nc.vector.memset(eps_t[:], LN_EPS)
        ident = consts.tile([P, P], F32)
        make_identity(nc, ident[:])
        tri_f = consts.tile([P, P], F32)
        nc.sync.dma_start(out=tri_f[:], in_=tri[:])
        tri_b = consts.tile([P, P], BF16)
        nc.vector.tensor_copy(out=tri_b[:], in_=tri_f[:])
        seg_sb = consts.tile([P, 2, FB], F32)
        nc.sync.dma_start(out=seg_sb[:], in_=seg.rearrange("s (fb p) -> p s fb", p=P))
        bev_t = consts.tile([1, 1], F32)
        nc.sync.dma_start(out=bev_t[:], in_=b_ev[:, None])

        # ---------------- helpers ----------------
        def load_h_chunk(gcol0, n):
            hf = hio.tile([P, FB, n], F32, tag="hf")
            for fb in range(FB):
                nc.sync.dma_start(out=hf[:, fb, :], in_=hdram[fb, :, gcol0:gcol0 + n])
            return hf

        def emit_ln(hf, n, g_vec, b_vec):
            """LN over feature dim for one chunk; returns hn [P, FB, n] f32r."""
            mu_ps = psum.tile([1, n], F32, tag="stat1")
            msq_ps = psum.tile([1, n], F32, tag="stat2")
            for fb in range(FB):
                hrf = small.tile([P, n], F32R, tag="hrf")
                nc.scalar.activation(out=hrf[:], in_=hf[:, fb, :], func=AF.Copy)
                nc.tensor.matmul(out=mu_ps[:], lhsT=ones_col[:], rhs=hrf[:],
                                 start=(fb == 0), stop=(fb == FB - 1))
                hsq = small.tile([P, n], F32R, tag="hsq")
                nc.scalar.activation(out=hsq[:], in_=hrf[:], func=AF.Square)
                nc.tensor.matmul(out=msq_ps[:], lhsT=ones_col[:], rhs=hsq[:],
                                 start=(fb == 0), stop=(fb == FB - 1))
            mu = small.tile([1, n], F32R, tag="mu")
            msq = small.tile([1, n], F32, tag="msq")
            nc.scalar.activation(out=mu[:], in_=mu_ps[:], func=AF.Copy, scale=1.0 / D)
            nc.scalar.activation(out=msq[:], in_=msq_ps[:], func=AF.Copy, scale=1.0 / D)
            var = small.tile([1, n], F32, tag="var")
            nc.vector.tensor_tensor(out=var[:], in0=mu[:], in1=mu[:], op=OP.mult)
            nc.vector.tensor_tensor(out=var[:], in0=msq[:], in1=var[:], op=OP.subtract)
            rstd = small.tile([1, n], F32R, tag="rstd")
            nc.scalar.activation(out=rstd[:], in_=var[:], func=AF.Sqrt, bias=eps_t[:])
            with nc.allow_low_precision(reason="rstd f32r feeds broadcast matmul"):
                nc.vector.reciprocal(out=rstd[:], in_=rstd[:])
            bc_mu = psum.tile([P, n], F32, tag="bcA")
            bc_r = psum.tile([P, n], F32, tag="bcB")
            nc.tensor.matmul(out=bc_mu[:], lhsT=ones_row[:], rhs=mu[:], start=True, stop=True)
            nc.tensor.matmul(out=bc_r[:], lhsT=ones_row[:], rhs=rstd[:], start=True, stop=True)
            hn = hnp.tile([P, FB, n], F32R, tag="hn")
            for fb in range(FB):
                tmp = small.tile([P, n], F32, tag="lntmp")
                nc.vector.tensor_tensor(out=tmp[:], in0=hf[:, fb, :], in1=bc_mu[:], op=OP.subtract)
                nc.vector.tensor_tensor(out=tmp[:], in0=tmp[:], in1=bc_r[:], op=OP.mult)
                nc.vector.tensor_scalar(out=hn[:, fb, :], in0=tmp[:],
                                        scalar1=g_vec[:, fb:fb + 1], scalar2=b_vec[:, fb:fb + 1],
                                        op0=OP.mult, op1=OP.add)
            return hn

        def writeback_fb(fb, gcol0, n, d_ps, bias_vec):
            """h[fb, :, cols] += d_ps + bias."""
            hres = hio.tile([P, n], F32, tag="hres")
            nc.sync.dma_start(out=hres[:], in_=hdram[fb, :, gcol0:gcol0 + n])
            hnew = hio.tile([P, n], F32, tag="hnew")
            nc.vector.tensor_scalar_add(out=hnew[:], in0=d_ps[:],
                                        scalar1=bias_vec[:, fb:fb + 1])
            nc.vector.tensor_tensor(out=hnew[:], in0=hnew[:], in1=hres[:], op=OP.add)
            nc.sync.dma_start(out=hdram[fb, :, gcol0:gcol0 + n], in_=hnew[:])

        def load_weight(dram_ap, kdim, mdim, tag, pool, dt=F32R):
            """DMA (kdim, mdim) weight -> [P, kdim//P, mdim] rounded tile."""
            ko = kdim // P
            wr = pool.tile([P, ko, mdim], dt, tag=tag)
            src = dram_ap.rearrange("(ko p) m -> p ko m", p=P)
            mstep = max(1, (2048 // max(ko, 1)))  # stage pieces <= [P, ko, mstep] (8KB)
            m0 = 0
            while m0 < mdim:
                m1 = min(m0 + mstep, mdim)
                wf = wstage.tile([P, ko, m1 - m0], F32, tag="stage")
                nc.sync.dma_start(out=wf[:], in_=src[:, :, m0:m1])
                if dt == F32R:
                    nc.gpsimd.tensor_copy(out=wr[:, :, m0:m1], in_=wf[:])
                else:
                    nc.vector.tensor_copy(out=wr[:, :, m0:m1], in_=wf[:])
                m0 = m1
            return wr

        def load_vec(dram_ap, nblk, tag):
            v = vecs.tile([P, nblk], F32, tag=tag)
            nc.sync.dma_start(out=v[:], in_=dram_ap.rearrange("(nb p) -> p nb", p=P))
            return v

        # ---------------- embedding ----------------
        for tt in range(TT // P):
            col0 = tt * P
            idx_sb = att.tile([P, 1], I32, tag="idx")
            nc.sync.dma_start(out=idx_sb[:], in_=ids[col0:col0 + P, :])
            g = att.tile([P, D], F32, tag="qa")
            nc.gpsimd.indirect_dma_start(
                out=g[:], out_offset=None, in_=emb[:],
                in_offset=bass.IndirectOffsetOnAxis(ap=idx_sb[:, :1], axis=0))
            ptile = att.tile([P, D], F32, tag="qb")
            r = 0
            while r < P:
                tok = col0 + r
                it, off = divmod(tok, L)
                if off < K:
                    ln = min(K - off, P - r)
                    nc.sync.dma_start(out=ptile[r:r + ln, :], in_=cpos[off:off + ln, :])
                else:
                    j = off - K
                    ln = min(W - j, P - r)
                    nc.sync.dma_start(out=ptile[r:r + ln, :], in_=spos[j:j + ln, :])
                r += ln
            nc.vector.tensor_tensor(out=g[:], in0=g[:], in1=ptile[:], op=OP.add)
            for fb in range(FB):
                tp = psum2.tile([P, P], F32, tag="mm")
                nc.tensor.transpose(out=tp[:], in_=g[:, fb * P:(fb + 1) * P], identity=ident[:])
                hout = att.tile([P, P], F32, tag="et")
                r = 0
                while r < P:
                    tok = col0 + r
                    it, off = divmod(tok, L)
                    s = 0 if off < K else 1
                    ln = (min(K - off, P - r)) if off < K else (min(L - off, P - r))
                    nc.vector.tensor_scalar_add(out=hout[:, r:r + ln], in0=tp[:, r:r + ln],
                                                scalar1=seg_sb[:, s, fb:fb + 1])
                    r += ln
                nc.sync.dma_start(out=hdram[fb, :, col0:col0 + P], in_=hout[:])

        if debug_tap == "emb":
            for fb in range(FB):
                nc.sync.dma_start(out=dbg_h[fb, :, :], in_=hdram[fb, :, :])

        # ---------------- transformer layers ----------------
        for l in range(n_layers):
            wq_s_r = load_weight(wq_s[l], D, DH, "wq_s", wqkv)
            wk_s_r = load_weight(wk_s[l], D, DH, "wk_s", wqkv)
            wv_s_r = load_weight(wv_s[l], D, DH, "wv_s", wqkv)
            wq_c_r = load_weight(wq_c[l], D, DH, "wq_c", wqkv)
            wk_c_r = load_weight(wk_c[l], D, DH, "wk_c", wqkv)
            wv_c_r = load_weight(wv_c[l], D, DH, "wv_c", wqkv)
            wo_r = load_weight(w_out[l], D, D, "wout", wqkv)
            g1 = load_vec(ln1_g[l], FB, "ln1g")
            b1 = load_vec(ln1_b[l], FB, "ln1b")
            bo = load_vec(b_out[l], FB, "bout")

            for item in range(ITEMS):
                base = item * L
                kaT = mega.tile([P, DHB, W], BF16, tag="mA")
                va = mega.tile([P, W // P, DH], F32R, tag="mB")
                kbT = att.tile([P, DHB, K], BF16, tag="kbT")
                vb = att.tile([P, DH], F32R, tag="vb")
                qc_a = att.tile([P, DHB, K], BF16, tag="qc_a")
                qc_b = att.tile([P, DHB, K], BF16, tag="qc_b")

                def project(hn, n, w_r, mdim, slice_fn):
                    for mo in range(mdim // P):
                        pj = psum2.tile([P, n], F32, tag="mm")
                        for ko in range(FB):
                            nc.tensor.matmul(
                                out=pj[:], lhsT=w_r[:, ko, mo * P:(mo + 1) * P],
                                rhs=hn[:, ko, :], start=(ko == 0), stop=(ko == FB - 1))
                        nc.vector.tensor_copy(out=slice_fn(mo), in_=pj[:])

                def attn_chunk(c, col0, n, hn_or_none):
                    if c == -1:
                        qa, qb = qc_a, qc_b
                    else:
                        qa = att.tile([P, DHB, n], BF16, tag="qa")
                        qb = att.tile([P, DHB, n], BF16, tag="qb")
                        project(hn_or_none, n, wq_s_r, DH, lambda mo: qa[:, mo, :])
                        project(hn_or_none, n, wq_c_r, DH, lambda mo: qb[:, mo, :])
                    n_kt = 16 if c == -1 else 4 * c + 4
                    oa0 = psum.tile([P, n], F32, tag="oa0")
                    oa1 = psum.tile([P, n], F32, tag="oa1")
                    suma = psum.tile([1, n], F32, tag="stat1")
                    for kt in range(n_kt):
                        lo = 0 if (c == -1 or kt < 4 * c) else 128 * (kt - 4 * c)
                        s_ps = psum2.tile([P, n - lo], F32, tag="mm")
                        for dhb in range(DHB):
                            nc.tensor.matmul(
                                out=s_ps[:], lhsT=kaT[:, dhb, kt * P:(kt + 1) * P],
                                rhs=qa[:, dhb, lo:n], start=(dhb == 0), stop=(dhb == DHB - 1))
                        e_t = att.tile([P, n], F32R, tag="et")
                        nc.scalar.activation(out=e_t[:, lo:n], in_=s_ps[:], func=AF.Exp,
                                             scale=SCALE)
                        if c != -1 and kt >= 4 * c:
                            nc.vector.tensor_tensor(out=e_t[:, lo:lo + P], in0=e_t[:, lo:lo + P],
                                                    in1=tri_f[:], op=OP.mult)
                        st, sp = kt == 0, kt == n_kt - 1
                        nc.tensor.matmul(out=suma[:, lo:n], lhsT=ones_col[:],
                                         rhs=e_t[:, lo:n], start=st, stop=sp)
                        nc.tensor.matmul(out=oa0[:, lo:n], lhsT=va[:, kt, 0:P],
                                         rhs=e_t[:, lo:n], start=st, stop=sp)
                        nc.tensor.matmul(out=oa1[:, lo:n], lhsT=va[:, kt, P:DH],
                                         rhs=e_t[:, lo:n], start=st, stop=sp)
                    # stream b (cache keys, full attention)
                    sb_ps = psum2.tile([K, n], F32, tag="mm")
                    for dhb in range(DHB):
                        nc.tensor.matmul(out=sb_ps[:], lhsT=kbT[:, dhb, :], rhs=qb[:, dhb, :],
                                         start=(dhb == 0), stop=(dhb == DHB - 1))
                    e_b = att.tile([K, n], F32R, tag="eb")
                    nc.scalar.activation(out=e_b[:], in_=sb_ps[:], func=AF.Exp, scale=SCALE)
                    sumb = psum.tile([1, n], F32, tag="stat2")
                    nc.tensor.matmul(out=sumb[:], lhsT=ones_col[:K, :], rhs=e_b[:],
                                     start=True, stop=True)
                    ob0 = psum2.tile([P, n], F32, tag="mm")
                    ob1 = psum2.tile([P, n], F32, tag="mm")
                    nc.tensor.matmul(out=ob0[:], lhsT=vb[:K, 0:P], rhs=e_b[:], start=True, stop=True)
                    nc.tensor.matmul(out=ob1[:], lhsT=vb[:K, P:DH], rhs=e_b[:], start=True, stop=True)
                    # normalize + concat
                    ra = small.tile([1, n], F32R, tag="ra")
                    rb = small.tile([1, n], F32R, tag="rb")
                    with nc.allow_low_precision(reason="softmax recip f32r feeds broadcast matmul"):
                        nc.vector.reciprocal(out=ra[:], in_=suma[:])
                        nc.vector.reciprocal(out=rb[:], in_=sumb[:])
                    bca_ps = psum.tile([P, n], F32, tag="bcA")
                    bcb_ps = psum.tile([P, n], F32, tag="bcB")
                    nc.tensor.matmul(out=bca_ps[:], lhsT=ones_row[:], rhs=ra[:], start=True, stop=True)
                    nc.tensor.matmul(out=bcb_ps[:], lhsT=ones_row[:], rhs=rb[:], start=True, stop=True)
                    bca = small.tile([P, n], F32, tag="bca")
                    bcb = small.tile([P, n], F32, tag="bcb")
                    nc.vector.tensor_copy(out=bca[:], in_=bca_ps[:])
                    nc.vector.tensor_copy(out=bcb[:], in_=bcb_ps[:])
                    ao = mega.tile([P, FB, n], F32R, tag="mI")
                    nc.vector.tensor_tensor(out=ao[:, 0, :], in0=oa0[:], in1=bca[:], op=OP.mult)
                    nc.vector.tensor_tensor(out=ao[:, 1, :], in0=oa1[:], in1=bca[:], op=OP.mult)
                    nc.vector.tensor_tensor(out=ao[:, 2, :], in0=ob0[:], in1=bcb[:], op=OP.mult)
                    nc.vector.tensor_tensor(out=ao[:, 3, :], in0=ob1[:], in1=bcb[:], op=OP.mult)
                    for fb in range(FB):
                        dp = psum2.tile([P, n], F32, tag="mm")
                        for ko in range(FB):
                            nc.tensor.matmul(out=dp[:], lhsT=wo_r[:, ko, fb * P:(fb + 1) * P],
                                             rhs=ao[:, ko, :], start=(ko == 0), stop=(ko == FB - 1))
                        writeback_fb(fb, base + col0, n, dp, bo)

                for (c, col0, n) in ITEM_CHUNKS:
                    hf = load_h_chunk(base + col0, n)
                    hn = emit_ln(hf, n, g1, b1)
                    if c == -1:
                        project(hn, n, wk_c_r, DH, lambda mo: kbT[:, mo, :])
                        project(hn, n, wq_s_r, DH, lambda mo: qc_a[:, mo, :])
                        project(hn, n, wq_c_r, DH, lambda mo: qc_b[:, mo, :])
                        vb_ps = psum2.tile([K, DH], F32, tag="mm")
                        for ko in range(FB):
                            nc.tensor.matmul(out=vb_ps[:], lhsT=hn[:, ko, :], rhs=wv_c_r[:, ko, :],
                                             start=(ko == 0), stop=(ko == FB - 1))
                        nc.vector.tensor_copy(out=vb[:K, :], in_=vb_ps[:])
                    else:
                        project(hn, n, wk_s_r, DH,
                                lambda mo: kaT[:, mo, 512 * c:512 * c + n])
                        for i in range(4):
                            kt = 4 * c + i
                            va_ps = psum2.tile([P, DH], F32, tag="mm")
                            for ko in range(FB):
                                nc.tensor.matmul(out=va_ps[:], lhsT=hn[:, ko, i * P:(i + 1) * P],
                                                 rhs=wv_s_r[:, ko, :], start=(ko == 0), stop=(ko == FB - 1))
                            nc.vector.tensor_copy(out=va[:, kt, :], in_=va_ps[:])
                        attn_chunk(c, col0, n, hn)
                attn_chunk(-1, 0, 64, None)

            # ---- FFN ----
            wf1h = []
            for h in range(2):
                w1h = load_weight(w_ff1[l][:, h * 1024:(h + 1) * 1024], D, 1024,
                                  ("mA", "mB")[h], mega)
                wf1h.append(w1h)
            wf2h = []
            for h in range(2):
                w2h = load_weight(w_ff2[l][h * 1024:(h + 1) * 1024, :], 1024, D,
                                  ("mC", "mD")[h], mega, dt=BF16)
                wf2h.append(w2h)
            g2 = load_vec(ln2_g[l], FB, "ln2g")
            b2 = load_vec(ln2_b[l], FB, "ln2b")
            bf1 = load_vec(b_ff1[l], DFF // P, "bff1")
            bf2 = load_vec(b_ff2[l], FB, "bff2")

            for (gc0, n) in GLOB_CHUNKS:
                hf = load_h_chunk(gc0, n)
                hn2 = emit_ln(hf, n, g2, b2)
                ffT = [mega.tile([P, 8, n], BF16, tag=t, name=f"ffT_{t}") for t in ("mI", "mF")]
                for h in range(2):
                    for mo in range(8):
                        fp = psum2.tile([P, n], F32, tag="mm")
                        for ko in range(FB):
                            nc.tensor.matmul(out=fp[:], lhsT=wf1h[h][:, ko, mo * P:(mo + 1) * P],
                                             rhs=hn2[:, ko, :], start=(ko == 0), stop=(ko == FB - 1))
                        nc.scalar.activation(out=ffT[h][:, mo, :], in_=fp[:], func=AF.Relu,
                                             bias=bf1[:, 8 * h + mo:8 * h + mo + 1])
                for fb in range(FB):
                    dp = psum2.tile([P, n], F32, tag="mm")
                    first = True
                    for h in range(2):
                        for ko in range(8):
                            nc.tensor.matmul(out=dp[:], lhsT=wf2h[h][:, ko, fb * P:(fb + 1) * P],
                                             rhs=ffT[h][:, ko, :], start=first,
                                             stop=(h == 1 and ko == 7))
                            first = False
                    writeback_fb(fb, gc0, n, dp, bf2)

            if debug_tap == ("layer", l):
                for fb in range(FB):
                    nc.sync.dma_start(out=dbg_h[fb, :, :], in_=hdram[fb, :, :])

        # ---------------- final LN + logits ----------------
        gF = load_vec(fin_g, FB, "fing")
        bF = load_vec(fin_b, FB, "finb")
        wev_r = load_weight(w_ev, D, 1, "wev", wqkv)
        for item in range(ITEMS):
            hf = load_h_chunk(item * L, K)
            hnF = emit_ln(hf, K, gF, bF)
            lg = psum2.tile([1, K], F32, tag="mm")
            for ko in range(FB):
                nc.tensor.matmul(out=lg[:], lhsT=wev_r[:, ko, :], rhs=hnF[:, ko, :],
                                 start=(ko == 0), stop=(ko == FB - 1))
            o_sb = small.tile([1, K], F32, tag="osb")
            nc.vector.tensor_scalar_add(out=o_sb[:], in0=lg[:], scalar1=bev_t[:])
            nc.sync.dma_start(out=out[item:item + 1, :], in_=o_sb[:])

    nc.finalize()
    return nc


def make_in_maps(inputs):
    f32 = lambda x: np.ascontiguousarray(np.asarray(x), dtype=np.float32)
    cache = np.asarray(inputs["cache"]).astype(np.int32)
    seq = np.asarray(inputs["seq"]).astype(np.int32)
    shared = {
        "item_embed": f32(inputs["item_embed"]),
        "cache_pos": f32(inputs["cache_pos_embed"]),
        "seq_pos": f32(inputs["seq_pos_embed"]),
        "seg": f32(inputs["segment_embed"]),
        "tri": np.triu(np.ones((P, P), np.float32)),
        "w_out": f32(inputs["w_out"]), "b_out": f32(inputs["b_out"]),
        "ln1_g": f32(inputs["ln1_g"]), "ln1_b": f32(inputs["ln1_b"]),
        "ln2_g": f32(inputs["ln2_g"]), "ln2_b": f32(inputs["ln2_b"]),
        "w_ff1": f32(inputs["w_ff1"]), "b_ff1": f32(inputs["b_ff1"]),
        "w_ff2": f32(inputs["w_ff2"]), "b_ff2": f32(inputs["b_ff2"]),
        "fin_g": f32(inputs["fin_g"]), "fin_b": f32(inputs["fin_b"]),
        "w_ev": f32(inputs["w_ev"]), "b_ev": f32(inputs["b_ev"]),
        "wq_s": f32(inputs["wq_s"]), "wk_s": f32(inputs["wk_s"]),
        "wv_s": f32(inputs["wv_s"]), "wq_c": f32(inputs["wq_c"]),
        "wk_c": f32(inputs["wk_c"]), "wv_c": f32(inputs["wv_c"]),
    }
    in_maps = []
    for core in range(NCORES):
        ids_l = []
        for item in range(ITEMS):
            b = core * ITEMS + item
            ids_l.append(np.concatenate([cache[b], seq[b]]))
        m = dict(shared)
        m["ids"] = np.concatenate(ids_l).reshape(TT, 1).astype(np.int32)
        in_maps.append(m)
    return in_maps


def kernel(**inputs) -> np.ndarray:
    if "nc" not in _CACHE:
        _CACHE["nc"] = build_nc()
    res = run_bass_kernel_spmd(_CACHE["nc"], make_in_maps(inputs), list(range(NCORES)))
    outs = [res.results[c]["out"] for c in range(NCORES)]
    return np.concatenate(outs, axis=0).astype(np.float32)


if __name__ == "__main__":
    import time
    t0 = time.time()
    build_nc()
    print(f"build+finalize: {time.time()-t0:.1f}s")
